# revision 1
# baseline (speedup 1.0000x reference)
"""Talking-heads attention (ViT-B/16-ish shapes) on 8 Trainium2 NeuronCores.

Problem: B=16, N=577, C=768, H=12 heads, d=64.
  qkv = x @ Wqkv.T ; logits = q k^T * scale ; pre-softmax head mix (Wpre);
  softmax ; post-softmax head mix (Wpost) ; out = (attn @ v) @ Wproj.T + b.

Distribution: pure data-parallel over batch, 2 batches per core, no
collectives.

Per-core design (all matmuls bf16 inputs, fp32 PSUM accumulation):
  - host pre-transposes x to [C, N] and pre-casts/packs all weights.
  - qkv:   q,k in [feat, tok] layout; v in [tok, feat] layout.
  - logits per head, K=64, two heads run concurrently via PE row groups.
  - talking-heads mixing runs as 120x120 block-diagonal matmuls in a packed
    layout [(h-major: p = 10h + n_i), m] over blocks of 10 query rows.
    The partition interleave that builds this layout is impossible in one
    SBUF->SBUF DMA (only the first AP dim may cross partitions), so the pack
    round-trips through a DRAM scratch laid out [b][p][m]: 12 block writes +
    1 packed read per query tile, all large affine DMAs.
  - softmax without max-subtraction (logits are small); exp on ScalarE with
    accum_out producing the row sums; normalize on VectorE (per-partition
    1/S); statistics in fp32.
  - post-mix is fused with the transpose AV needs: E-tile is the stationary
    operand, block-diag Wpost^T the moving one, giving P'^T[m, (10g+n)] in
    PSUM directly.
  - AV consumes P'^T with a strided free AP per head; head pairs run
    concurrently via PE column groups. Output lands in [feat, tok] layout,
    which feeds the final projection without any transpose.
"""

import numpy as np
import ml_dtypes

import concourse.bass as bass
import concourse.mybir as mybir
from concourse import bacc
from concourse.tile import TileContext
from concourse.bass_utils import run_bass_kernel_spmd

BF16 = ml_dtypes.bfloat16

B, N, C, H = 16, 577, 768, 12
D = C // H                 # 64
NCORES = 8
BPC = B // NCORES          # batches per core = 2
NPAD = 600                 # padded query-token count (5 qtiles of 120)
QT = 5                     # query tiles
QTW = 120                  # rows per query tile
NI = 10                    # query rows per packed block
BPQ = QTW // NI            # blocks per qtile = 12
FT = C // 128              # feature tiles = 6
MT = [128, 128, 128, 128, 65]   # key-token tiles (sum 577)
MOF = [0, 128, 256, 384, 512]

_NC_CACHE = {}


def _build_nc(debug=False):
    nc = bacc.Bacc("TRN2", target_bir_lowering=False)
    dt = mybir.dt
    dbg = {}
    if debug:
        dbg["q"] = nc.dram_tensor("dbg_q", [128, FT, NPAD], dt.bfloat16, kind="ExternalOutput")
        dbg["k"] = nc.dram_tensor("dbg_k", [128, FT, N], dt.bfloat16, kind="ExternalOutput")
        dbg["v"] = nc.dram_tensor("dbg_v", [128, len(MT), C], dt.bfloat16, kind="ExternalOutput")
        dbg["lnat"] = nc.dram_tensor("dbg_lnat", [QTW, H, N], dt.bfloat16, kind="ExternalOutput")
        dbg["lpk"] = nc.dram_tensor("dbg_lpk", [QTW, BPQ, N], dt.bfloat16, kind="ExternalOutput")
        dbg["e"] = nc.dram_tensor("dbg_e", [QTW, BPQ, N], dt.bfloat16, kind="ExternalOutput")
        dbg["pt"] = nc.dram_tensor("dbg_pt", [128, len(MT), BPQ, QTW], dt.bfloat16, kind="ExternalOutput")
        dbg["o"] = nc.dram_tensor("dbg_o", [128, FT, NPAD], dt.bfloat16, kind="ExternalOutput")

    xT = nc.dram_tensor("xT", [BPC, C, NPAD], dt.bfloat16, kind="ExternalInput")
    wqT = nc.dram_tensor("wqT", [C, C], dt.bfloat16, kind="ExternalInput")
    wkT = nc.dram_tensor("wkT", [C, C], dt.bfloat16, kind="ExternalInput")
    wvT = nc.dram_tensor("wvT", [C, C], dt.bfloat16, kind="ExternalInput")
    wpT = nc.dram_tensor("wpT", [C, C], dt.bfloat16, kind="ExternalInput")
    bdpre = nc.dram_tensor("bdpre", [QTW, QTW], dt.bfloat16, kind="ExternalInput")
    bdpostT = nc.dram_tensor("bdpostT", [QTW, QTW], dt.bfloat16, kind="ExternalInput")
    bias = nc.dram_tensor("bias", [C], dt.float32, kind="ExternalInput")
    y = nc.dram_tensor("y", [BPC, N, C], dt.float32, kind="ExternalOutput")
    # packed-logits scratch, laid out [batch][qtile][block][p = 10h + n_i][m]
    pk = nc.dram_tensor("pk", [BPC, QT, BPQ, QTW, N], dt.bfloat16, kind="Internal")

    with TileContext(nc) as tc:
        with (
            tc.tile_pool(name="consts", bufs=1) as consts,
            tc.tile_pool(name="qkv", bufs=1) as qkvp,
            tc.tile_pool(name="lnatp", bufs=3) as lnatp,
            tc.tile_pool(name="stage", bufs=2) as stage,
            tc.tile_pool(name="mid", bufs=1) as mid,
            tc.tile_pool(name="lpkp", bufs=3) as lpkp,
            tc.tile_pool(name="outp", bufs=2) as outp,
            tc.tile_pool(name="ps_big", bufs=2, space="PSUM") as ps_big,
            tc.tile_pool(name="ps_pm", bufs=2, space="PSUM") as ps_pm,
        ):
            # ---- constants ----
            wq_sb = consts.tile([128, FT, C], dt.bfloat16, tag="wq")
            wk_sb = consts.tile([128, FT, C], dt.bfloat16, tag="wk")
            wv_sb = consts.tile([128, FT, C], dt.bfloat16, tag="wv")
            wp_sb = consts.tile([128, FT, C], dt.bfloat16, tag="wp")
            for w_sb, w_dr, eng in ((wq_sb, wqT, nc.scalar), (wk_sb, wkT, nc.gpsimd),
                                    (wv_sb, wvT, nc.scalar), (wp_sb, wpT, nc.gpsimd)):
                eng.dma_start(out=w_sb[:], in_=w_dr.rearrange("(t p) f -> p t f", p=128))
            bdpre_sb = consts.tile([QTW, QTW], dt.bfloat16, tag="bdpre")
            nc.scalar.dma_start(out=bdpre_sb[:], in_=bdpre[:])
            bdpostT_sb = consts.tile([QTW, QTW], dt.bfloat16, tag="bdpostT")
            nc.gpsimd.dma_start(out=bdpostT_sb[:], in_=bdpostT[:])
            bias_sb = consts.tile([128, C], dt.float32, tag="bias")
            nc.scalar.dma_start(
                out=bias_sb[:],
                in_=bass.AP(tensor=bias[:].tensor, offset=0, ap=[[0, 128], [1, C]]),
            )

            for bi in range(BPC):
                # ---- load x^T ----
                xT_sb = qkvp.tile([128, FT, NPAD], dt.bfloat16, tag="xT")
                nc.sync.dma_start(
                    out=xT_sb[:], in_=xT[bi].rearrange("(t p) n -> p t n", p=128)
                )

                # ---- qkv projection ----
                q_sb = qkvp.tile([128, FT, NPAD], dt.bfloat16, tag="q")
                k_sb = qkvp.tile([128, FT, N], dt.bfloat16, tag="k")
                v_sb = qkvp.tile([128, len(MT), C], dt.bfloat16, tag="v")
                for ft in range(FT):  # q, k: [feat, tok]
                    for dst, w_sb, ntok in ((q_sb, wq_sb, NPAD), (k_sb, wk_sb, N)):
                        ps = ps_big.tile([128, NPAD], dt.float32, tag="big")
                        for kc in range(FT):
                            for lo, hi in ((0, 512), (512, ntok)):
                                nc.tensor.matmul(
                                    out=ps[:, lo:hi],
                                    lhsT=w_sb[:, kc, ft * 128:(ft + 1) * 128],
                                    rhs=xT_sb[:, kc, lo:hi],
                                    start=(kc == 0), stop=(kc == FT - 1),
                                )
                        if ft % 2 == 0:
                            nc.vector.tensor_copy(out=dst[:, ft, :], in_=ps[:, 0:ntok])
                        else:
                            nc.scalar.copy(out=dst[:, ft, :], in_=ps[:, 0:ntok])
                for mt in range(len(MT)):  # v: [tok, feat]
                    ps = ps_big.tile([128, C], dt.float32, tag="big")
                    mw = MT[mt]
                    for kc in range(FT):
                        for lo, hi in ((0, 512), (512, C)):
                            nc.tensor.matmul(
                                out=ps[0:mw, lo:hi],
                                lhsT=xT_sb[:, kc, MOF[mt]:MOF[mt] + mw],
                                rhs=wv_sb[:, kc, lo:hi],
                                start=(kc == 0), stop=(kc == FT - 1),
                            )
                    eng = nc.vector if mt % 2 == 0 else nc.scalar
                    if mt % 2 == 0:
                        eng.tensor_copy(out=v_sb[0:mw, mt, :], in_=ps[0:mw, 0:C])
                    else:
                        eng.copy(out=v_sb[0:mw, mt, :], in_=ps[0:mw, 0:C])

                o_sb = qkvp.tile([128, FT, NPAD], dt.bfloat16, tag="o")
                if debug and bi == 0:
                    nc.sync.dma_start(out=dbg["q"][:], in_=q_sb[:])
                    nc.sync.dma_start(out=dbg["k"][:], in_=k_sb[:])
                    for mt in range(len(MT)):
                        nc.sync.dma_start(out=dbg["v"][0:MT[mt], mt, :],
                                          in_=v_sb[0:MT[mt], mt, :])

                def emit_logits(qt):
                    """logits + evac + pack-write + pack-read kickoff."""
                    q0 = qt * QTW
                    l_nat = lnatp.tile([QTW, H, N], dt.bfloat16, tag="lnat")
                    for hp in range(H // 2):
                        ps0 = ps_big.tile([QTW, N], dt.float32, tag="big")
                        ps1 = ps_big.tile([QTW, N], dt.float32, tag="big")
                        for sub, ps in ((0, ps0), (1, ps1)):
                            pbase = 64 * sub
                            for lo, hi in ((0, 512), (512, N)):
                                nc.tensor.matmul(
                                    out=ps[:, lo:hi],
                                    lhsT=q_sb[pbase:pbase + 64, hp, q0:q0 + QTW],
                                    rhs=k_sb[pbase:pbase + 64, hp, lo:hi],
                                )
                        nc.vector.tensor_copy(out=l_nat[:, 2 * hp, :], in_=ps0[:])
                        nc.scalar.copy(out=l_nat[:, 2 * hp + 1, :], in_=ps1[:])
                    # pack round-trip: writes split 2:1 over SP/Pool queues;
                    # the first read chunk goes on Pool now, the rest are
                    # issued later (in emit_middle) on ACT to avoid blocking
                    # the exp stream behind a waiting DMA (in-order queues).
                    for b in range(BPQ):
                        eng = nc.gpsimd if b % 2 == 1 else nc.sync
                        eng.dma_start(
                            out=pk[bi, qt, b].rearrange("(h n) m -> n h m", n=NI),
                            in_=l_nat[NI * b:NI * (b + 1), :, :],
                        )
                    l_pk = lpkp.tile([QTW, BPQ, N], dt.bfloat16, tag="lpk")
                    nc.gpsimd.dma_start(
                        out=l_pk[:, 0:4, :],
                        in_=pk[bi, qt, 0:4].rearrange("b p m -> p b m"),
                    )
                    if debug and bi == 0 and qt == 0:
                        nc.sync.dma_start(out=dbg["lnat"][:], in_=l_nat[:])
                        nc.sync.dma_start(out=dbg["lpk"][:], in_=l_pk[:])
                    return l_pk

                def emit_rest_of_read(qt2):
                    for j in (1, 2):
                        nc.scalar.dma_start(
                            out=lpks[qt2][:, 4 * j:4 * (j + 1), :],
                            in_=pk[bi, qt2, 4 * j:4 * (j + 1)].rearrange("b p m -> p b m"),
                        )

                def emit_middle(qt, l_pk):
                    """premix, softmax, postmix-T, AV, proj for one qtile."""
                    q0 = qt * QTW
                    e_sb = mid.tile([QTW, BPQ, N], dt.bfloat16, tag="e")
                    s_sb = stage.tile([QTW, BPQ], dt.float32, tag="s")
                    for b in range(BPQ):
                        ps = ps_pm.tile([QTW, N], dt.float32, tag="pm")
                        for lo, hi in ((0, 512), (512, N)):
                            nc.tensor.matmul(
                                out=ps[:, lo:hi], lhsT=bdpre_sb[:], rhs=l_pk[:, b, lo:hi]
                            )
                        nc.scalar.activation(
                            out=e_sb[:, b, :], in_=ps[:],
                            func=mybir.ActivationFunctionType.Exp,
                            accum_out=s_sb[:, b:b + 1],
                        )
                    if qt + 1 in lpks:
                        emit_rest_of_read(qt + 1)
                    sinv = stage.tile([QTW, BPQ], dt.float32, tag="sinv")
                    nc.vector.reciprocal(out=sinv[:], in_=s_sb[:])
                    for b in range(BPQ):
                        nc.vector.tensor_scalar_mul(
                            e_sb[:, b, :], e_sb[:, b, :], sinv[:, b:b + 1]
                        )
                    # fused postmix+transpose: P'^T[m, 10g+n] in PSUM
                    pt_sb = mid.tile([128, len(MT), BPQ, QTW], dt.bfloat16, tag="pt")
                    for mt in range(len(MT)):
                        mw = MT[mt]
                        for bg in range(BPQ // 4):
                            ps = ps_pm.tile([128, 4 * QTW], dt.float32, tag="pm")
                            for sl in range(4):
                                b = 4 * bg + sl
                                nc.tensor.matmul(
                                    out=ps[0:mw, sl * QTW:(sl + 1) * QTW],
                                    lhsT=e_sb[:, b, MOF[mt]:MOF[mt] + mw],
                                    rhs=bdpostT_sb[:],
                                )
                            eng_v = (mt + bg) % 2 == 0
                            dst = pt_sb[0:mw, mt, 4 * bg:4 * (bg + 1), :]
                            if eng_v:
                                nc.vector.tensor_copy(out=dst, in_=ps[0:mw, 0:4 * QTW])
                            else:
                                nc.scalar.copy(out=dst, in_=ps[0:mw, 0:4 * QTW])
                    if debug and bi == 0 and qt == 0:
                        nc.sync.dma_start(out=dbg["e"][:], in_=e_sb[:])
                        for mt in range(len(MT)):
                            nc.sync.dma_start(out=dbg["pt"][0:MT[mt], mt],
                                              in_=pt_sb[0:MT[mt], mt])
                    # AV: head pairs via PE column groups
                    for gp in range(H // 2):
                        ps = ps_big.tile([128, QTW], dt.float32, tag="big")
                        for sub in range(2):
                            g = 2 * gp + sub
                            for mt in range(len(MT)):
                                mw = MT[mt]
                                nc.tensor.matmul(
                                    out=ps[64 * sub:64 * (sub + 1), :],
                                    lhsT=v_sb[0:mw, mt, 64 * g:64 * (g + 1)],
                                    rhs=pt_sb[0:mw, mt, :, NI * g:NI * (g + 1)],
                                    start=(mt == 0), stop=(mt == len(MT) - 1),
                                    skip_group_check=True,
                                )
                        if gp % 2 == 0:
                            nc.vector.tensor_copy(out=o_sb[:, gp, q0:q0 + QTW], in_=ps[:])
                        else:
                            nc.scalar.copy(out=o_sb[:, gp, q0:q0 + QTW], in_=ps[:])
                    # output projection + bias for this qtile
                    ps = ps_big.tile([QTW, C], dt.float32, tag="big")
                    for kc in range(FT):
                        for lo, hi in ((0, 512), (512, C)):
                            nc.tensor.matmul(
                                out=ps[:, lo:hi],
                                lhsT=o_sb[:, kc, q0:q0 + QTW],
                                rhs=wp_sb[:, kc, lo:hi],
                                start=(kc == 0), stop=(kc == FT - 1),
                            )
                    out_sb = outp.tile([QTW, C], dt.float32, tag="out")
                    nc.vector.tensor_tensor(
                        out=out_sb[:], in0=ps[:], in1=bias_sb[0:QTW, :],
                        op=mybir.AluOpType.add,
                    )
                    rows = min(N - q0, QTW)
                    nc.sync.dma_start(out=y[bi, q0:q0 + rows, :], in_=out_sb[0:rows, :])

                # software pipeline: logits of qt+2 issue before middle of qt
                lpks = {}
                lpks[0] = emit_logits(0)
                emit_rest_of_read(0)
                lpks[1] = emit_logits(1)
                for qt in range(QT):
                    if qt + 2 < QT:
                        lpks[qt + 2] = emit_logits(qt + 2)
                    emit_middle(qt, lpks[qt])
                    del lpks[qt]
                if debug and bi == 0:
                    nc.sync.dma_start(out=dbg["o"][:], in_=o_sb[:])
    nc.compile()
    return nc


def _host_prep(x, Wqkv, Wproj, bproj, Wpre, Wpost):
    scale = D ** -0.5
    Wq = (Wqkv[0:C] * scale).T        # [C, C] lhsT for q (scale folded)
    Wk = Wqkv[C:2 * C].T
    Wv = Wqkv[2 * C:3 * C].T
    Wp = Wproj.T
    # h-major packed-block mixing matrices (p = 10*h + n_i)
    eye = np.eye(NI, dtype=np.float32)
    # bdpre[(10h+ni), (10g+nj)] = Wpre[g, h] * (ni == nj)
    bdpre = np.einsum("gh,ij->higj", Wpre.astype(np.float32), eye).reshape(QTW, QTW)
    # bdpostT[(10g+ni), (10g'+nj)] = Wpost[g', g] * (ni == nj)
    bdpostT = np.einsum("pg,ij->gipj", Wpost.astype(np.float32), eye).reshape(QTW, QTW)

    xT = np.zeros((B, C, NPAD), dtype=BF16)
    xT[:, :, 0:N] = np.ascontiguousarray(x.transpose(0, 2, 1)).astype(BF16)
    return {
        "xT": xT,
        "wqT": np.ascontiguousarray(Wq).astype(BF16),
        "wkT": np.ascontiguousarray(Wk).astype(BF16),
        "wvT": np.ascontiguousarray(Wv).astype(BF16),
        "wpT": np.ascontiguousarray(Wp).astype(BF16),
        "bdpre": bdpre.astype(BF16),
        "bdpostT": bdpostT.astype(BF16),
        "bias": bproj.astype(np.float32),
    }


def kernel(x, Wqkv, Wproj, bproj, Wpre, Wpost):
    x = np.asarray(x, dtype=np.float32)
    Wqkv = np.asarray(Wqkv, dtype=np.float32)
    Wproj = np.asarray(Wproj, dtype=np.float32)
    bproj = np.asarray(bproj, dtype=np.float32)
    Wpre = np.asarray(Wpre, dtype=np.float32)
    Wpost = np.asarray(Wpost, dtype=np.float32)

    host = _host_prep(x, Wqkv, Wproj, bproj, Wpre, Wpost)
    if "nc" not in _NC_CACHE:
        _NC_CACHE["nc"] = _build_nc()
    nc = _NC_CACHE["nc"]

    shared = {k: host[k] for k in
              ("wqT", "wkT", "wvT", "wpT", "bdpre", "bdpostT", "bias")}
    in_maps = []
    for core in range(NCORES):
        m = dict(shared)
        m["xT"] = host["xT"][core * BPC:(core + 1) * BPC]
        in_maps.append(m)

    res = run_bass_kernel_spmd(nc, in_maps, core_ids=list(range(NCORES)))
    out = np.concatenate([np.asarray(r["y"]) for r in res.results], axis=0)
    return out.astype(np.float32)

